# revision 1
# baseline (speedup 1.0000x reference)
"""DeepSeekMoE block on 8 Trainium2 NeuronCores.

Sharding: expert-parallel — core e owns expert e's FFN (up_w[e]/down_w[e]);
tokens are dispatched to expert cores by host-side top-2 gating (the gate
matmul is 0.03% of total FLOPs).  The shared expert is token-parallel:
core e also runs the shared FFN for tokens [e*256, (e+1)*256).

Device kernel per core (SPMD):
  hact = gelu(up_w[e].T-tiles @ xT + up_b[e])        # [I, cap] layout
  eoT  = 0.1 * (down_w[e]-tiles @ hact + down_b[e])  # [H, cap]
  same for the shared expert on its 256-token slice.
Matmuls run in bf16 (fp32 PSUM accumulate); set DTYPE="f32r" for
full-rate fp32 (2x DMA traffic, ~10x lower error).

Host: gating/top-k (fp64 scores, fp32 combine weights), scatter-add of the
two expert contributions per token + shared path, row max-abs normalize.
"""
import sys
sys.path.insert(0, '/opt/trn_rl_repo')
import numpy as np
from contextlib import ExitStack

H = 1024
I = 4096
E = 8
TOPK = 2
B, S = 2, 1024
T = B * S            # 2048 tokens
CAP = 544            # routed-token capacity per expert core (max count is 542)
TS = T // E          # shared-expert tokens per core = 256
HC = H // 128        # 8 h-chunks
IC = I // 128        # 32 i-chunks
DTYPE = "bf16"       # "bf16" | "f32r"
PHASES = ("routed", "shared")

if DTYPE == "bf16":
    BLK_R = (512, 32)
    BLK_S = (256,)
else:
    BLK_R = (288, 256)   # f32r needs moving dim >=256 for full rate
    BLK_S = (256,)

_COMPILED = {}


def _build_nc():
    from concourse import bacc, tile, mybir

    F32 = mybir.dt.float32
    CDT = mybir.dt.bfloat16 if DTYPE == "bf16" else mybir.dt.float32r
    GELU = mybir.ActivationFunctionType.Gelu
    IDENT = mybir.ActivationFunctionType.Identity

    nc = bacc.Bacc("TRN2", target_bir_lowering=False, debug=False, num_devices=E)

    xT_d = nc.dram_tensor("xT", [128, HC * CAP], CDT, kind="ExternalInput")
    xsT_d = nc.dram_tensor("xsT", [128, HC * TS], CDT, kind="ExternalInput")
    upw_d = nc.dram_tensor("upw", [128, IC * HC * 128], CDT, kind="ExternalInput")
    dnw_d = nc.dram_tensor("dnw", [128, HC * IC * 128], CDT, kind="ExternalInput")
    supw_d = nc.dram_tensor("supw", [128, IC * HC * 128], CDT, kind="ExternalInput")
    sdnw_d = nc.dram_tensor("sdnw", [128, HC * IC * 128], CDT, kind="ExternalInput")
    upb_d = nc.dram_tensor("upb", [128, IC], F32, kind="ExternalInput")
    supb_d = nc.dram_tensor("supb", [128, IC], F32, kind="ExternalInput")
    dnb_d = nc.dram_tensor("dnb", [128, HC], F32, kind="ExternalInput")
    sdnb_d = nc.dram_tensor("sdnb", [128, HC], F32, kind="ExternalInput")
    eoT_d = nc.dram_tensor("eoT", [HC, 128, CAP], F32, kind="ExternalOutput")
    soT_d = nc.dram_tensor("soT", [HC, 128, TS], F32, kind="ExternalOutput")

    with tile.TileContext(nc) as tc, ExitStack() as ctx:
        pool = ctx.enter_context(tc.tile_pool(name="sbuf", bufs=1))
        uwpool = ctx.enter_context(tc.tile_pool(name="uwstream", bufs=4))
        dwpool = ctx.enter_context(tc.tile_pool(name="dwstream", bufs=3))
        hpool_r = ctx.enter_context(tc.tile_pool(name="hact_r", bufs=IC))
        hpool_s = ctx.enter_context(tc.tile_pool(name="hact_s", bufs=IC))
        opool = ctx.enter_context(tc.tile_pool(name="outs", bufs=6))
        upps = ctx.enter_context(tc.tile_pool(name="upps", bufs=2, space="PSUM"))
        dnps = ctx.enter_context(tc.tile_pool(name="dnps", bufs=2, space="PSUM"))

        # resident activations + biases
        xT_t = pool.tile([128, HC * CAP], CDT, tag="xT")
        for hc in range(HC):   # chunked so the first matmuls start sooner
            nc.sync.dma_start(xT_t[:, hc * CAP:(hc + 1) * CAP],
                              xT_d.ap()[:, hc * CAP:(hc + 1) * CAP])
        xsT_t = pool.tile([128, HC * TS], CDT, tag="xsT")
        nc.sync.dma_start(xsT_t[:], xsT_d.ap()[:])
        upb_t = pool.tile([128, IC], F32, tag="upb")
        nc.sync.dma_start(upb_t[:], upb_d.ap()[:])
        supb_t = pool.tile([128, IC], F32, tag="supb")
        nc.sync.dma_start(supb_t[:], supb_d.ap()[:])
        dnb_t = pool.tile([128, HC], F32, tag="dnb")
        nc.sync.dma_start(dnb_t[:], dnb_d.ap()[:])
        sdnb_t = pool.tile([128, HC], F32, tag="sdnb")
        nc.sync.dma_start(sdnb_t[:], sdnb_d.ap()[:])

        def ffn(x_t, w_up_d, w_dn_d, b_up_t, b_dn_t, out_d, ntok, blocks, hpool):
            """One expert FFN over `ntok` token columns of x_t ([128, HC*ntok])."""
            # --- up projection + gelu: hact[ic] = gelu(up_w.T @ x + b) ---
            hacts = []
            for ic in range(IC):
                uw = uwpool.tile([128, HC * 128], CDT, tag="upw")
                nc.sync.dma_start(
                    uw[:], w_up_d.ap()[:, ic * HC * 128:(ic + 1) * HC * 128])
                ht = hpool.tile([128, ntok], CDT, tag="hact")
                t0 = 0
                for nb in blocks:
                    ps = upps.tile([128, nb], F32, tag="upps")
                    for hc in range(HC):
                        nc.tensor.matmul(
                            ps[:],
                            uw[:, hc * 128:(hc + 1) * 128],
                            x_t[:, hc * ntok + t0: hc * ntok + t0 + nb],
                            start=(hc == 0), stop=(hc == HC - 1),
                        )
                    if DTYPE == "bf16":
                        nc.scalar.activation(
                            ht[:, t0:t0 + nb], ps[:], GELU, bias=b_up_t[:, ic:ic + 1])
                    else:
                        # ScalarE cannot round to f32r (HW garbage) -> gelu to f32
                        # staging tile, DVE copy performs the f32r rounding.
                        g32 = opool.tile([128, nb], F32, tag="g32")
                        nc.scalar.activation(
                            g32[:], ps[:], GELU, bias=b_up_t[:, ic:ic + 1])
                        nc.vector.tensor_copy(ht[:, t0:t0 + nb], g32[:])
                    t0 += nb
                hacts.append(ht)

            # --- down projection: out[hb] = 0.1 * (dn_w.T @ hact + b) ---
            for hb in range(HC):
                dw = dwpool.tile([128, IC * 128], CDT, tag="dnw")
                nc.sync.dma_start(
                    dw[:], w_dn_d.ap()[:, hb * IC * 128:(hb + 1) * IC * 128])
                t0 = 0
                for nb in blocks:
                    ps = dnps.tile([128, nb], F32, tag="dnps")
                    for ic in range(IC):
                        nc.tensor.matmul(
                            ps[:],
                            dw[:, ic * 128:(ic + 1) * 128],
                            hacts[ic][:, t0:t0 + nb],
                            start=(ic == 0), stop=(ic == IC - 1),
                        )
                    ot = opool.tile([128, nb], F32, tag="out")
                    nc.scalar.activation(
                        ot[:], ps[:], IDENT, bias=b_dn_t[:, hb:hb + 1], scale=0.1)
                    nc.sync.dma_start(out_d.ap()[hb, :, t0:t0 + nb], ot[:])
                    t0 += nb

        if "routed" in PHASES:
            ffn(xT_t, upw_d, dnw_d, upb_t, dnb_t, eoT_d, CAP, BLK_R, hpool_r)
        if "shared" in PHASES:
            ffn(xsT_t, supw_d, sdnw_d, supb_t, sdnb_t, soT_d, TS, BLK_S, hpool_s)

    nc.compile()
    return nc


def _get_compiled():
    if "nc" not in _COMPILED:
        _COMPILED["nc"] = _build_nc()
    return _COMPILED["nc"]


def _np_cdt():
    if DTYPE == "bf16":
        import ml_dtypes
        return np.dtype(ml_dtypes.bfloat16)
    return np.dtype(np.float32)


def _pack_weight(w):
    """[K, N] -> [128, (N/128 chunks) x (K/128 subtiles) x 128] stream layout."""
    kdim, ndim = w.shape
    kc, nchunk = kdim // 128, ndim // 128
    return np.ascontiguousarray(
        w.reshape(kc, 128, nchunk, 128).transpose(1, 2, 0, 3)
    ).reshape(128, nchunk * kc * 128).astype(_np_cdt())


def _pack_tokens(xsel, cap):
    """[n, H] tokens -> [128, HC*cap] transposed h-chunked layout, zero pad."""
    n = xsel.shape[0]
    arr = np.zeros((128, HC, cap), np.float32)
    if n:
        arr[:, :, :n] = xsel.T.reshape(HC, 128, n).transpose(1, 0, 2)
    return np.ascontiguousarray(arr).reshape(128, HC * cap).astype(_np_cdt())


def _pack_bias(b, scale=1.0):
    """[N] -> [128, N/128] per-partition layout."""
    return np.ascontiguousarray(
        (np.asarray(b, np.float32) * scale).reshape(-1, 128).T.astype(np.float32))


def kernel(x, gate_w, bias, up_w, up_b, down_w, down_b,
           sw_up, sb_up, sw_down, sb_down):
    from concourse.bass_utils import run_bass_kernel_spmd

    x = np.asarray(x, np.float32)
    xf = x.reshape(T, H)

    # ---- host gating (fp64 scores for a stable top-k, fp32 combine weights)
    z64 = xf.astype(np.float64) @ np.asarray(gate_w, np.float64) \
        + np.asarray(bias, np.float64)
    scores64 = 1.0 / (1.0 + np.exp(-z64))
    top_idx = np.argsort(-scores64, axis=-1, kind="stable")[:, :TOPK]
    tsc = scores64[np.arange(T)[:, None], top_idx].astype(np.float32)
    wts = tsc / (tsc.sum(-1, keepdims=True) + np.float32(1e-6))   # [T, 2]

    # ---- token dispatch
    tok_lists = [np.where((top_idx == e).any(-1))[0] for e in range(E)]
    for e, tl in enumerate(tok_lists):
        if len(tl) > CAP:
            raise RuntimeError(f"expert {e} overflow: {len(tl)} > CAP={CAP}")

    supw = _pack_weight(np.asarray(sw_up, np.float32))
    sdnw = _pack_weight(np.asarray(sw_down, np.float32))
    supb = _pack_bias(sb_up)
    sdnb = _pack_bias(sb_down, scale=0.1)

    in_maps = []
    for e in range(E):
        in_maps.append({
            "xT": _pack_tokens(xf[tok_lists[e]], CAP),
            "xsT": _pack_tokens(xf[e * TS:(e + 1) * TS], TS),
            "upw": _pack_weight(np.asarray(up_w[e], np.float32)),
            "dnw": _pack_weight(np.asarray(down_w[e], np.float32)),
            "supw": supw,
            "sdnw": sdnw,
            "upb": _pack_bias(up_b[e]),
            "supb": supb,
            "dnb": _pack_bias(down_b[e], scale=0.1),
            "sdnb": sdnb,
        })

    nc = _get_compiled()
    res = run_bass_kernel_spmd(nc, in_maps, list(range(E)))

    # ---- host combine: scatter-add expert outputs, add shared, normalize
    out = np.zeros((T, H), np.float32)
    for e in range(E):
        soT = np.asarray(res.results[e]["soT"], np.float32)   # [HC, 128, TS]
        out[e * TS:(e + 1) * TS] = soT.reshape(H, TS).T
    for e in range(E):
        tl = tok_lists[e]
        if len(tl) == 0:
            continue
        eoT = np.asarray(res.results[e]["eoT"], np.float32)   # [HC, 128, CAP]
        eo = eoT.reshape(H, CAP)[:, :len(tl)].T               # [n, H]
        we = np.where(top_idx[tl, 0] == e, wts[tl, 0], wts[tl, 1]).astype(np.float32)
        out[tl] += we[:, None] * eo

    out /= (np.abs(out).max(-1, keepdims=True) + np.float32(1e-6))
    return out.reshape(B, S, H)



# revision 8
# speedup vs baseline: 1.2883x; 1.2883x over previous
"""DeepSeekMoE block on 8 Trainium2 NeuronCores.

Sharding:
  * Routed experts: expert-parallel - core e owns expert e's FFN
    (up_w[e]/down_w[e]); tokens are dispatched by host-side top-2 gating
    (the gate matmul is 0.03% of total FLOPs).  Capacity 544, processed
    as two 272-column blocks so every matmul is stream-bound (no N=32
    tail blocks, LDWEIGHTS fully hidden).
  * Shared expert: tensor-parallel over the intermediate dim - core e
    owns sw_up[:, e*512:(e+1)*512] / sw_down[e*512:(e+1)*512, :] and
    processes ALL 2048 tokens for its slice with N=512 matmuls.  The 8
    partial outputs (each 0.1 * hact_slice @ sw_down_slice) are summed
    on host.  This cuts per-core weight DMA from 32MB to 18MB vs
    replicating the shared expert.

Device kernel per core (SPMD), all matmuls bf16 with fp32 PSUM:
  routed: hact = gelu(up_w[e].T @ xT + up_b[e]);  eoT = 0.1*(dn_w[e].T @ hact + dn_b[e])
  shared: hs   = gelu(sup_slice.T @ x_all + b);   spT = 0.1*(sdn_slice.T @ hs)

Host: gating/top-k (fp64 scores), scatter-add of expert contributions,
sum of shared partials + shared down bias, row max-abs normalize.
"""
import os
import sys
sys.path.insert(0, '/opt/trn_rl_repo')
import numpy as np
from contextlib import ExitStack

# debug knob: KMOE_SEQ=1 runs each PSUM accumulation group to completion
# before starting its sibling block (no A/B interleave)
SEQ_GROUPS = os.environ.get("KMOE_SEQ", "0") == "1"

H = 1024
I = 4096
E = 8
TOPK = 2
B, S = 2, 1024
T = B * S            # 2048 tokens
CAP = 544            # routed-token capacity per expert core (max count is 542)
BLK = CAP // 2       # 272-column blocks (<=512 fp32 PSUM bank limit)
HC = H // 128        # 8 h-chunks
IC = I // 128        # 32 i-chunks (routed experts)
ISL = I // E         # 512: shared-expert intermediate slice per core
ICS = ISL // 128     # 4 i-chunks (shared slice)
TB = 512             # shared-expert token block
NTP = T // (2 * TB)  # 2 token-block pairs

_COMPILED = {}


def _build_nc():
    from concourse import bacc, tile, mybir

    F32 = mybir.dt.float32
    CDT = mybir.dt.bfloat16
    GELU = mybir.ActivationFunctionType.Gelu
    IDENT = mybir.ActivationFunctionType.Identity

    nc = bacc.Bacc("TRN2", target_bir_lowering=False, debug=False, num_devices=E)

    xT_d = nc.dram_tensor("xT", [128, HC * CAP], CDT, kind="ExternalInput")
    xa_d = nc.dram_tensor("xa", [128, HC * T], CDT, kind="ExternalInput")
    upw_d = nc.dram_tensor("upw", [128, IC * HC * 128], CDT, kind="ExternalInput")
    dnw_d = nc.dram_tensor("dnw", [128, HC * IC * 128], CDT, kind="ExternalInput")
    suw_d = nc.dram_tensor("suw", [128, ICS * HC * 128], CDT, kind="ExternalInput")
    sdw_d = nc.dram_tensor("sdw", [128, HC * ICS * 128], CDT, kind="ExternalInput")
    upb_d = nc.dram_tensor("upb", [128, IC], F32, kind="ExternalInput")
    supb_d = nc.dram_tensor("supb", [128, ICS], F32, kind="ExternalInput")
    dnb_d = nc.dram_tensor("dnb", [128, HC], F32, kind="ExternalInput")
    eoT_d = nc.dram_tensor("eoT", [HC, 128, CAP], F32, kind="ExternalOutput")
    spT_d = nc.dram_tensor("spT", [HC, 128, T], CDT, kind="ExternalOutput")

    with tile.TileContext(nc) as tc, ExitStack() as ctx:
        pool = ctx.enter_context(tc.tile_pool(name="persist", bufs=1))
        wup = ctx.enter_context(tc.tile_pool(name="wup", bufs=3))
        wdn = ctx.enter_context(tc.tile_pool(name="wdn", bufs=3))
        hr = ctx.enter_context(tc.tile_pool(name="hr", bufs=IC))
        hsp = ctx.enter_context(tc.tile_pool(name="hsp", bufs=ICS))
        orp = ctx.enter_context(tc.tile_pool(name="orp", bufs=3))
        osp = ctx.enter_context(tc.tile_pool(name="osp", bufs=3))
        pup = ctx.enter_context(tc.tile_pool(name="pup", bufs=2, space="PSUM"))
        pdn = ctx.enter_context(tc.tile_pool(name="pdn", bufs=2, space="PSUM"))

        # ---- input DMAs, ordered for the startup ramp: the first up-matmul
        # group needs xT + the first two up-weight i-chunks only.
        xT_t = pool.tile([128, HC * CAP], CDT, tag="xT")
        nc.sync.dma_start(xT_t[:], xT_d.ap()[:])

        uw_tiles = []
        uw0 = wup.tile([128, 4 * HC * 128], CDT, tag="uw")
        nc.sync.dma_start(uw0[:, :2048], upw_d.ap()[:, 0:2048])
        nc.sync.dma_start(uw0[:, 2048:], upw_d.ap()[:, 2048:4096])
        uw_tiles.append(uw0)
        upb_t = pool.tile([128, IC], F32, tag="upb")
        nc.sync.dma_start(upb_t[:], upb_d.ap()[:])
        uw1 = wup.tile([128, 4 * HC * 128], CDT, tag="uw")
        nc.sync.dma_start(uw1[:], upw_d.ap()[:, 4096:8192])
        uw_tiles.append(uw1)

        # needed only from the shared phase on; issued early, transfers
        # overlap the routed phase.
        xa_t = pool.tile([128, HC * T], CDT, tag="xa")
        half = HC * T // 2
        nc.sync.dma_start(xa_t[:, :half], xa_d.ap()[:, :half])
        nc.sync.dma_start(xa_t[:, half:], xa_d.ap()[:, half:])
        suw_t = pool.tile([128, ICS * HC * 128], CDT, tag="suw")
        nc.sync.dma_start(suw_t[:], suw_d.ap()[:])
        sdw_t = pool.tile([128, HC * ICS * 128], CDT, tag="sdw")
        nc.sync.dma_start(sdw_t[:], sdw_d.ap()[:])
        supb_t = pool.tile([128, ICS], F32, tag="supb")
        nc.sync.dma_start(supb_t[:], supb_d.ap()[:])
        dnb_t = pool.tile([128, HC], F32, tag="dnb")
        nc.sync.dma_start(dnb_t[:], dnb_d.ap()[:])

        def mm_pair(psA, psB, wfn, rA, rB, n):
            """Two accumulation groups over n contraction chunks."""
            if SEQ_GROUPS:
                for k in range(n):
                    nc.tensor.matmul(psA, wfn(k), rA(k), start=(k == 0), stop=(k == n - 1))
                for k in range(n):
                    nc.tensor.matmul(psB, wfn(k), rB(k), start=(k == 0), stop=(k == n - 1))
            else:
                for k in range(n):
                    w = wfn(k)
                    nc.tensor.matmul(psA, w, rA(k), start=(k == 0), stop=(k == n - 1))
                    nc.tensor.matmul(psB, w, rB(k), start=(k == 0), stop=(k == n - 1))

        # ---- routed up: hact[ic] = gelu(up_w.T @ x + b), two 272-col blocks
        hacts = []
        for icx in range(IC):
            g, off = divmod(icx, 4)
            if off == 0 and g >= 2:
                uwg = wup.tile([128, 4 * HC * 128], CDT, tag="uw", name=f"uwg{g}")
                nc.sync.dma_start(uwg[:], upw_d.ap()[:, g * 4096:(g + 1) * 4096])
                uw_tiles.append(uwg)
            uwt = uw_tiles[g]
            psA = pup.tile([128, 512], F32, tag="uA")
            psB = pup.tile([128, 512], F32, tag="uB")
            ht = hr.tile([128, CAP], CDT, tag="hact")
            mm_pair(psA[:, :BLK], psB[:, :BLK],
                    lambda hc, uwt=uwt, off=off: uwt[:, off * 1024 + hc * 128: off * 1024 + (hc + 1) * 128],
                    lambda hc: xT_t[:, hc * CAP: hc * CAP + BLK],
                    lambda hc: xT_t[:, hc * CAP + BLK: (hc + 1) * CAP], HC)
            nc.scalar.activation(ht[:, :BLK], psA[:, :BLK], GELU, bias=upb_t[:, icx:icx + 1])
            nc.scalar.activation(ht[:, BLK:], psB[:, :BLK], GELU, bias=upb_t[:, icx:icx + 1])
            hacts.append(ht)

        # ---- routed down: eoT[hb] = 0.1 * (dn_w.T @ hact + b)
        for hb in range(HC):
            dwt = wdn.tile([128, IC * 128], CDT, tag="dw")
            nc.sync.dma_start(dwt[:], dnw_d.ap()[:, hb * IC * 128:(hb + 1) * IC * 128])
            psA = pdn.tile([128, 512], F32, tag="dA")
            psB = pdn.tile([128, 512], F32, tag="dB")
            mm_pair(psA[:, :BLK], psB[:, :BLK],
                    lambda icx, dwt=dwt: dwt[:, icx * 128:(icx + 1) * 128],
                    lambda icx: hacts[icx][:, :BLK],
                    lambda icx: hacts[icx][:, BLK:], IC)
            ot = orp.tile([128, CAP], F32, tag="or")
            nc.scalar.activation(ot[:, :BLK], psA[:, :BLK], IDENT,
                                 bias=dnb_t[:, hb:hb + 1], scale=0.1)
            nc.scalar.activation(ot[:, BLK:], psB[:, :BLK], IDENT,
                                 bias=dnb_t[:, hb:hb + 1], scale=0.1)
            nc.sync.dma_start(eoT_d.ap()[hb, :, :], ot[:])

        # ---- shared up: hs[ic] = gelu(suw_slice.T @ x_all + b), N=512 blocks
        hs_tiles = []
        for icx in range(ICS):
            ht = hsp.tile([128, T], CDT, tag="hs")
            for tp in range(NTP):
                psA = pup.tile([128, 512], F32, tag="uA")
                psB = pup.tile([128, 512], F32, tag="uB")
                mm_pair(psA[:], psB[:],
                        lambda hc, icx=icx: suw_t[:, icx * 1024 + hc * 128: icx * 1024 + (hc + 1) * 128],
                        lambda hc, tp=tp: xa_t[:, hc * T + tp * 2 * TB: hc * T + tp * 2 * TB + TB],
                        lambda hc, tp=tp: xa_t[:, hc * T + tp * 2 * TB + TB: hc * T + (tp + 1) * 2 * TB], HC)
                o0 = tp * 2 * TB
                nc.scalar.activation(ht[:, o0: o0 + TB], psA[:], GELU,
                                     bias=supb_t[:, icx:icx + 1])
                nc.scalar.activation(ht[:, o0 + TB: o0 + 2 * TB], psB[:], GELU,
                                     bias=supb_t[:, icx:icx + 1])
            hs_tiles.append(ht)

        # ---- shared down: spT[hb] = 0.1 * (sdw_slice.T @ hs)  (bias on host)
        for hb in range(HC):
            for tp in range(NTP):
                psA = pdn.tile([128, 512], F32, tag="dA")
                psB = pdn.tile([128, 512], F32, tag="dB")
                mm_pair(psA[:], psB[:],
                        lambda icx, hb=hb: sdw_t[:, hb * ICS * 128 + icx * 128: hb * ICS * 128 + (icx + 1) * 128],
                        lambda icx, tp=tp: hs_tiles[icx][:, tp * 2 * TB: tp * 2 * TB + TB],
                        lambda icx, tp=tp: hs_tiles[icx][:, tp * 2 * TB + TB: (tp + 1) * 2 * TB], ICS)
                ot = osp.tile([128, 2 * TB], CDT, tag="os")
                nc.scalar.activation(ot[:, :TB], psA[:], IDENT, scale=0.1)
                nc.scalar.activation(ot[:, TB:], psB[:], IDENT, scale=0.1)
                nc.sync.dma_start(spT_d.ap()[hb, :, tp * 2 * TB:(tp + 1) * 2 * TB], ot[:])

    nc.compile()
    return nc


def _get_compiled():
    if "nc" not in _COMPILED:
        _COMPILED["nc"] = _build_nc()
    return _COMPILED["nc"]


def _np_cdt():
    import ml_dtypes
    return np.dtype(ml_dtypes.bfloat16)


def _pack_weight(w):
    """[K, N] -> [128, (N/128 chunks) x (K/128 subtiles) x 128] stream layout."""
    kdim, ndim = w.shape
    kc, nchunk = kdim // 128, ndim // 128
    return np.ascontiguousarray(
        w.reshape(kc, 128, nchunk, 128).transpose(1, 2, 0, 3)
    ).reshape(128, nchunk * kc * 128).astype(_np_cdt())


def _pack_tokens(xsel, cap):
    """[n, H] tokens -> [128, HC*cap] transposed h-chunked layout, zero pad."""
    n = xsel.shape[0]
    arr = np.zeros((128, HC, cap), np.float32)
    if n:
        arr[:, :, :n] = xsel.T.reshape(HC, 128, n).transpose(1, 0, 2)
    return np.ascontiguousarray(arr).reshape(128, HC * cap).astype(_np_cdt())


def _pack_bias(b, scale=1.0):
    """[N] -> [128, N/128] per-partition layout."""
    return np.ascontiguousarray(
        (np.asarray(b, np.float32) * scale).reshape(-1, 128).T.astype(np.float32))


def kernel(x, gate_w, bias, up_w, up_b, down_w, down_b,
           sw_up, sb_up, sw_down, sb_down):
    from concourse.bass_utils import run_bass_kernel_spmd

    x = np.asarray(x, np.float32)
    xf = x.reshape(T, H)

    # ---- host gating (fp64 scores for a stable top-k, fp32 combine weights)
    z64 = xf.astype(np.float64) @ np.asarray(gate_w, np.float64) \
        + np.asarray(bias, np.float64)
    scores64 = 1.0 / (1.0 + np.exp(-z64))
    top_idx = np.argsort(-scores64, axis=-1, kind="stable")[:, :TOPK]
    tsc = scores64[np.arange(T)[:, None], top_idx].astype(np.float32)
    wts = tsc / (tsc.sum(-1, keepdims=True) + np.float32(1e-6))   # [T, 2]

    # ---- token dispatch
    tok_lists = [np.where((top_idx == e).any(-1))[0] for e in range(E)]
    for e, tl in enumerate(tok_lists):
        if len(tl) > CAP:
            raise RuntimeError(f"expert {e} overflow: {len(tl)} > CAP={CAP}")

    xa = _pack_tokens(xf, T)
    in_maps = []
    for e in range(E):
        in_maps.append({
            "xT": _pack_tokens(xf[tok_lists[e]], CAP),
            "xa": xa,
            "upw": _pack_weight(np.asarray(up_w[e], np.float32)),
            "dnw": _pack_weight(np.asarray(down_w[e], np.float32)),
            "suw": _pack_weight(np.asarray(sw_up[:, e * ISL:(e + 1) * ISL], np.float32)),
            "sdw": _pack_weight(np.asarray(sw_down[e * ISL:(e + 1) * ISL, :], np.float32)),
            "upb": _pack_bias(up_b[e]),
            "supb": _pack_bias(np.asarray(sb_up, np.float32)[e * ISL:(e + 1) * ISL]),
            "dnb": _pack_bias(down_b[e], scale=0.1),
        })

    nc = _get_compiled()
    res = run_bass_kernel_spmd(nc, in_maps, list(range(E)))

    # ---- host combine: shared partial sum + bias, expert scatter-add, normalize
    sp = np.zeros((H, T), np.float32)
    for e in range(E):
        sp += np.asarray(res.results[e]["spT"], np.float32).reshape(H, T)
    out = sp.T + np.float32(0.1) * np.asarray(sb_down, np.float32)[None, :]

    for e in range(E):
        tl = tok_lists[e]
        if len(tl) == 0:
            continue
        eoT = np.asarray(res.results[e]["eoT"], np.float32)   # [HC, 128, CAP]
        eo = eoT.reshape(H, CAP)[:, :len(tl)].T               # [n, H]
        we = np.where(top_idx[tl, 0] == e, wts[tl, 0], wts[tl, 1]).astype(np.float32)
        out[tl] += we[:, None] * eo

    out /= (np.abs(out).max(-1, keepdims=True) + np.float32(1e-6))
    return out.reshape(B, S, H)


# revision 11
# speedup vs baseline: 1.3780x; 1.0696x over previous
"""DeepSeekMoE block on 8 Trainium2 NeuronCores.

Sharding:
  * Routed experts: expert-parallel - core e owns expert e's FFN
    (up_w[e]/down_w[e]); tokens are dispatched by host-side top-2 gating
    (the gate matmul is 0.03% of total FLOPs).  Capacity 544, processed
    as two 272-column blocks so every matmul is stream-bound (no N=32
    tail blocks, LDWEIGHTS fully hidden).
  * Shared expert: tensor-parallel over the intermediate dim - core e
    owns sw_up[:, e*512:(e+1)*512] / sw_down[e*512:(e+1)*512, :] and
    processes ALL 2048 tokens for its slice with N=512 matmuls.  The 8
    partial outputs (each 0.1 * hact_slice @ sw_down_slice) are summed
    on host.  This cuts per-core weight DMA from 32MB to 18MB vs
    replicating the shared expert.

Device kernel per core (SPMD), all matmuls bf16 with fp32 PSUM:
  routed: hact = gelu(up_w[e].T @ xT + up_b[e]);  eoT = 0.1*(dn_w[e].T @ hact + dn_b[e])
  shared: hs   = gelu(sup_slice.T @ x_all + b);   spT = 0.1*(sdn_slice.T @ hs)

Host: gating/top-k (fp64 scores), scatter-add of expert contributions,
sum of shared partials + shared down bias, row max-abs normalize.
"""
import os
import sys
sys.path.insert(0, '/opt/trn_rl_repo')
import numpy as np
from contextlib import ExitStack

# debug knob: KMOE_SEQ=1 runs each PSUM accumulation group to completion
# before starting its sibling block (no A/B interleave)
SEQ_GROUPS = os.environ.get("KMOE_SEQ", "0") == "1"

H = 1024
I = 4096
E = 8
TOPK = 2
B, S = 2, 1024
T = B * S            # 2048 tokens
CAP = 544            # routed-token capacity per expert core (max count is 542)
BLK = CAP // 2       # 272-column blocks (<=512 fp32 PSUM bank limit)
HC = H // 128        # 8 h-chunks
IC = I // 128        # 32 i-chunks (routed experts)
ISL = I // E         # 512: shared-expert intermediate slice per core
ICS = ISL // 128     # 4 i-chunks (shared slice)
TB = 512             # shared-expert token block
NTP = T // (2 * TB)  # 2 token-block pairs

_COMPILED = {}


def _build_nc():
    from concourse import bacc, tile, mybir

    F32 = mybir.dt.float32
    CDT = mybir.dt.bfloat16
    GELU = mybir.ActivationFunctionType.Gelu
    IDENT = mybir.ActivationFunctionType.Identity

    nc = bacc.Bacc("TRN2", target_bir_lowering=False, debug=False, num_devices=E)

    xT_d = nc.dram_tensor("xT", [128, HC * CAP], CDT, kind="ExternalInput")
    xa_d = nc.dram_tensor("xa", [128, HC * T], CDT, kind="ExternalInput")
    upw_d = nc.dram_tensor("upw", [128, IC * HC * 128], CDT, kind="ExternalInput")
    dnw_d = nc.dram_tensor("dnw", [128, HC * IC * 128], CDT, kind="ExternalInput")
    suw_d = nc.dram_tensor("suw", [128, ICS * HC * 128], CDT, kind="ExternalInput")
    sdw_d = nc.dram_tensor("sdw", [128, HC * ICS * 128], CDT, kind="ExternalInput")
    upb_d = nc.dram_tensor("upb", [128, IC], F32, kind="ExternalInput")
    supb_d = nc.dram_tensor("supb", [128, ICS], F32, kind="ExternalInput")
    dnb_d = nc.dram_tensor("dnb", [128, HC], F32, kind="ExternalInput")
    eoT_d = nc.dram_tensor("eoT", [HC, 128, CAP], F32, kind="ExternalOutput")
    spT_d = nc.dram_tensor("spT", [HC, 128, T], CDT, kind="ExternalOutput")

    with tile.TileContext(nc) as tc, ExitStack() as ctx:
        pool = ctx.enter_context(tc.tile_pool(name="persist", bufs=1))
        wup = ctx.enter_context(tc.tile_pool(name="wup", bufs=3))
        wdn = ctx.enter_context(tc.tile_pool(name="wdn", bufs=3))
        hr = ctx.enter_context(tc.tile_pool(name="hr", bufs=IC))
        hsp = ctx.enter_context(tc.tile_pool(name="hsp", bufs=ICS))
        orp = ctx.enter_context(tc.tile_pool(name="orp", bufs=3))
        osp = ctx.enter_context(tc.tile_pool(name="osp", bufs=3))
        pup = ctx.enter_context(tc.tile_pool(name="pup", bufs=2, space="PSUM"))
        pdn = ctx.enter_context(tc.tile_pool(name="pdn", bufs=2, space="PSUM"))

        # ---- input DMAs, ordered for the startup ramp: the first up-matmul
        # group needs xT + the first up-weight i-chunks only; everything the
        # shared phase needs is deferred past the routed-up weight stream.
        xT_t = pool.tile([128, HC * CAP], CDT, tag="xT")
        xhalf = HC * CAP // 2
        nc.sync.dma_start(xT_t[:, :xhalf], xT_d.ap()[:, :xhalf])
        nc.sync.dma_start(xT_t[:, xhalf:], xT_d.ap()[:, xhalf:])

        uw_tiles = []
        uw0 = wup.tile([128, 4 * HC * 128], CDT, tag="uw")
        nc.sync.dma_start(uw0[:, :2048], upw_d.ap()[:, 0:2048])
        nc.sync.dma_start(uw0[:, 2048:], upw_d.ap()[:, 2048:4096])
        uw_tiles.append(uw0)
        upb_t = pool.tile([128, IC], F32, tag="upb")
        nc.sync.dma_start(upb_t[:], upb_d.ap()[:])
        for g in (1, 2):
            uwg = wup.tile([128, 4 * HC * 128], CDT, tag="uw", name=f"uwp{g}")
            nc.sync.dma_start(uwg[:], upw_d.ap()[:, g * 4096:(g + 1) * 4096])
            uw_tiles.append(uwg)

        # PE warm-up: ~32 dummy matmuls on a zeroed tile keep the PE busy
        # through the HAM activity window while the first weights stream in,
        # so the real matmul stream starts at the full 2.4 GHz clock.
        wz_t = pool.tile([128, 128], CDT, tag="wz")
        nc.gpsimd.memset(wz_t[:], 0.0)
        psW = pup.tile([128, 512], F32, tag="uA")
        for _ in range(32):
            nc.tensor.matmul(psW[:, :128], wz_t[:], wz_t[:], start=True, stop=True)

        def mm_pair(psA, psB, wfn, rA, rB, n):
            """Two accumulation groups over n contraction chunks."""
            if SEQ_GROUPS:
                for k in range(n):
                    nc.tensor.matmul(psA, wfn(k), rA(k), start=(k == 0), stop=(k == n - 1))
                for k in range(n):
                    nc.tensor.matmul(psB, wfn(k), rB(k), start=(k == 0), stop=(k == n - 1))
            else:
                for k in range(n):
                    w = wfn(k)
                    nc.tensor.matmul(psA, w, rA(k), start=(k == 0), stop=(k == n - 1))
                    nc.tensor.matmul(psB, w, rB(k), start=(k == 0), stop=(k == n - 1))

        # ---- routed up: hact[ic] = gelu(up_w.T @ x + b), two 272-col blocks
        hacts = []
        for icx in range(IC):
            g, off = divmod(icx, 4)
            if off == 0 and g >= 3:
                uwg = wup.tile([128, 4 * HC * 128], CDT, tag="uw", name=f"uwg{g}")
                nc.sync.dma_start(uwg[:], upw_d.ap()[:, g * 4096:(g + 1) * 4096])
                uw_tiles.append(uwg)
            uwt = uw_tiles[g]
            psA = pup.tile([128, 512], F32, tag="uA")
            psB = pup.tile([128, 512], F32, tag="uB")
            ht = hr.tile([128, CAP], CDT, tag="hact")
            mm_pair(psA[:, :BLK], psB[:, :BLK],
                    lambda hc, uwt=uwt, off=off: uwt[:, off * 1024 + hc * 128: off * 1024 + (hc + 1) * 128],
                    lambda hc: xT_t[:, hc * CAP: hc * CAP + BLK],
                    lambda hc: xT_t[:, hc * CAP + BLK: (hc + 1) * CAP], HC)
            nc.scalar.activation(ht[:, :BLK], psA[:, :BLK], GELU, bias=upb_t[:, icx:icx + 1])
            nc.scalar.activation(ht[:, BLK:], psB[:, :BLK], GELU, bias=upb_t[:, icx:icx + 1])
            hacts.append(ht)

        # shared-phase inputs: issued after the routed-up weight stream so
        # they never starve it (needed only ~100us later).
        xa_t = pool.tile([128, HC * T], CDT, tag="xa")
        half = HC * T // 2
        nc.sync.dma_start(xa_t[:, :half], xa_d.ap()[:, :half])
        nc.sync.dma_start(xa_t[:, half:], xa_d.ap()[:, half:])
        suw_t = pool.tile([128, ICS * HC * 128], CDT, tag="suw")
        nc.sync.dma_start(suw_t[:], suw_d.ap()[:])
        sdw_t = pool.tile([128, HC * ICS * 128], CDT, tag="sdw")
        nc.sync.dma_start(sdw_t[:], sdw_d.ap()[:])
        supb_t = pool.tile([128, ICS], F32, tag="supb")
        nc.sync.dma_start(supb_t[:], supb_d.ap()[:])
        dnb_t = pool.tile([128, HC], F32, tag="dnb")
        nc.sync.dma_start(dnb_t[:], dnb_d.ap()[:])

        # ---- routed down: eoT[hb] = 0.1 * (dn_w.T @ hact + b)
        for hb in range(HC):
            dwt = wdn.tile([128, IC * 128], CDT, tag="dw")
            nc.sync.dma_start(dwt[:], dnw_d.ap()[:, hb * IC * 128:(hb + 1) * IC * 128])
            psA = pdn.tile([128, 512], F32, tag="dA")
            psB = pdn.tile([128, 512], F32, tag="dB")
            mm_pair(psA[:, :BLK], psB[:, :BLK],
                    lambda icx, dwt=dwt: dwt[:, icx * 128:(icx + 1) * 128],
                    lambda icx: hacts[icx][:, :BLK],
                    lambda icx: hacts[icx][:, BLK:], IC)
            ot = orp.tile([128, CAP], F32, tag="or")
            nc.scalar.activation(ot[:, :BLK], psA[:, :BLK], IDENT,
                                 bias=dnb_t[:, hb:hb + 1], scale=0.1)
            nc.scalar.activation(ot[:, BLK:], psB[:, :BLK], IDENT,
                                 bias=dnb_t[:, hb:hb + 1], scale=0.1)
            nc.sync.dma_start(eoT_d.ap()[hb, :, :], ot[:])

        # ---- shared up: hs[ic] = gelu(suw_slice.T @ x_all + b), N=512 blocks
        hs_tiles = []
        for icx in range(ICS):
            ht = hsp.tile([128, T], CDT, tag="hs")
            for tp in range(NTP):
                psA = pup.tile([128, 512], F32, tag="uA")
                psB = pup.tile([128, 512], F32, tag="uB")
                mm_pair(psA[:], psB[:],
                        lambda hc, icx=icx: suw_t[:, icx * 1024 + hc * 128: icx * 1024 + (hc + 1) * 128],
                        lambda hc, tp=tp: xa_t[:, hc * T + tp * 2 * TB: hc * T + tp * 2 * TB + TB],
                        lambda hc, tp=tp: xa_t[:, hc * T + tp * 2 * TB + TB: hc * T + (tp + 1) * 2 * TB], HC)
                o0 = tp * 2 * TB
                nc.scalar.activation(ht[:, o0: o0 + TB], psA[:], GELU,
                                     bias=supb_t[:, icx:icx + 1])
                nc.scalar.activation(ht[:, o0 + TB: o0 + 2 * TB], psB[:], GELU,
                                     bias=supb_t[:, icx:icx + 1])
            hs_tiles.append(ht)

        # ---- shared down: spT[hb] = 0.1 * (sdw_slice.T @ hs)  (bias on host)
        for hb in range(HC):
            for tp in range(NTP):
                psA = pdn.tile([128, 512], F32, tag="dA")
                psB = pdn.tile([128, 512], F32, tag="dB")
                mm_pair(psA[:], psB[:],
                        lambda icx, hb=hb: sdw_t[:, hb * ICS * 128 + icx * 128: hb * ICS * 128 + (icx + 1) * 128],
                        lambda icx, tp=tp: hs_tiles[icx][:, tp * 2 * TB: tp * 2 * TB + TB],
                        lambda icx, tp=tp: hs_tiles[icx][:, tp * 2 * TB + TB: (tp + 1) * 2 * TB], ICS)
                ot = osp.tile([128, 2 * TB], CDT, tag="os")
                nc.scalar.activation(ot[:, :TB], psA[:], IDENT, scale=0.1)
                nc.scalar.activation(ot[:, TB:], psB[:], IDENT, scale=0.1)
                nc.sync.dma_start(spT_d.ap()[hb, :, tp * 2 * TB:(tp + 1) * 2 * TB], ot[:])

    nc.compile()
    return nc


def _get_compiled():
    if "nc" not in _COMPILED:
        _COMPILED["nc"] = _build_nc()
    return _COMPILED["nc"]


def _np_cdt():
    import ml_dtypes
    return np.dtype(ml_dtypes.bfloat16)


def _pack_weight(w):
    """[K, N] -> [128, (N/128 chunks) x (K/128 subtiles) x 128] stream layout."""
    kdim, ndim = w.shape
    kc, nchunk = kdim // 128, ndim // 128
    return np.ascontiguousarray(
        w.reshape(kc, 128, nchunk, 128).transpose(1, 2, 0, 3)
    ).reshape(128, nchunk * kc * 128).astype(_np_cdt())


def _pack_tokens(xsel, cap):
    """[n, H] tokens -> [128, HC*cap] transposed h-chunked layout, zero pad."""
    n = xsel.shape[0]
    arr = np.zeros((128, HC, cap), np.float32)
    if n:
        arr[:, :, :n] = xsel.T.reshape(HC, 128, n).transpose(1, 0, 2)
    return np.ascontiguousarray(arr).reshape(128, HC * cap).astype(_np_cdt())


def _pack_bias(b, scale=1.0):
    """[N] -> [128, N/128] per-partition layout."""
    return np.ascontiguousarray(
        (np.asarray(b, np.float32) * scale).reshape(-1, 128).T.astype(np.float32))


def kernel(x, gate_w, bias, up_w, up_b, down_w, down_b,
           sw_up, sb_up, sw_down, sb_down):
    from concourse.bass_utils import run_bass_kernel_spmd

    x = np.asarray(x, np.float32)
    xf = x.reshape(T, H)

    # ---- host gating (fp64 scores for a stable top-k, fp32 combine weights)
    z64 = xf.astype(np.float64) @ np.asarray(gate_w, np.float64) \
        + np.asarray(bias, np.float64)
    scores64 = 1.0 / (1.0 + np.exp(-z64))
    top_idx = np.argsort(-scores64, axis=-1, kind="stable")[:, :TOPK]
    tsc = scores64[np.arange(T)[:, None], top_idx].astype(np.float32)
    wts = tsc / (tsc.sum(-1, keepdims=True) + np.float32(1e-6))   # [T, 2]

    # ---- token dispatch
    tok_lists = [np.where((top_idx == e).any(-1))[0] for e in range(E)]
    for e, tl in enumerate(tok_lists):
        if len(tl) > CAP:
            raise RuntimeError(f"expert {e} overflow: {len(tl)} > CAP={CAP}")

    xa = _pack_tokens(xf, T)
    in_maps = []
    for e in range(E):
        in_maps.append({
            "xT": _pack_tokens(xf[tok_lists[e]], CAP),
            "xa": xa,
            "upw": _pack_weight(np.asarray(up_w[e], np.float32)),
            "dnw": _pack_weight(np.asarray(down_w[e], np.float32)),
            "suw": _pack_weight(np.asarray(sw_up[:, e * ISL:(e + 1) * ISL], np.float32)),
            "sdw": _pack_weight(np.asarray(sw_down[e * ISL:(e + 1) * ISL, :], np.float32)),
            "upb": _pack_bias(up_b[e]),
            "supb": _pack_bias(np.asarray(sb_up, np.float32)[e * ISL:(e + 1) * ISL]),
            "dnb": _pack_bias(down_b[e], scale=0.1),
        })

    nc = _get_compiled()
    res = run_bass_kernel_spmd(nc, in_maps, list(range(E)))

    # ---- host combine: shared partial sum + bias, expert scatter-add, normalize
    sp = np.zeros((H, T), np.float32)
    for e in range(E):
        sp += np.asarray(res.results[e]["spT"], np.float32).reshape(H, T)
    out = sp.T + np.float32(0.1) * np.asarray(sb_down, np.float32)[None, :]

    for e in range(E):
        tl = tok_lists[e]
        if len(tl) == 0:
            continue
        eoT = np.asarray(res.results[e]["eoT"], np.float32)   # [HC, 128, CAP]
        eo = eoT.reshape(H, CAP)[:, :len(tl)].T               # [n, H]
        we = np.where(top_idx[tl, 0] == e, wts[tl, 0], wts[tl, 1]).astype(np.float32)
        out[tl] += we[:, None] * eo

    out /= (np.abs(out).max(-1, keepdims=True) + np.float32(1e-6))
    return out.reshape(B, S, H)


# revision 14
# speedup vs baseline: 1.3907x; 1.0092x over previous
"""DeepSeekMoE block on 8 Trainium2 NeuronCores.

Sharding:
  * Routed experts: expert-parallel - core e owns expert e's FFN
    (up_w[e]/down_w[e]); tokens are dispatched by host-side top-2 gating
    (the gate matmul is 0.03% of total FLOPs).  Capacity 544, processed
    as two 272-column blocks so every matmul is stream-bound (no N=32
    tail blocks, LDWEIGHTS fully hidden).
  * Shared expert: tensor-parallel over the intermediate dim - core e
    owns sw_up[:, e*512:(e+1)*512] / sw_down[e*512:(e+1)*512, :] and
    processes ALL 2048 tokens for its slice with N=512 matmuls.  The 8
    partial outputs (each 0.1 * hact_slice @ sw_down_slice) are summed
    on host.  This cuts per-core weight DMA from 32MB to 18MB vs
    replicating the shared expert.

Device kernel per core (SPMD), all matmuls bf16 with fp32 PSUM:
  routed: hact = gelu(up_w[e].T @ xT + up_b[e]);  eoT = 0.1*(dn_w[e].T @ hact + dn_b[e])
  shared: hs   = gelu(sup_slice.T @ x_all + b);   spT = 0.1*(sdn_slice.T @ hs)

Host: gating/top-k (fp64 scores), scatter-add of expert contributions,
sum of shared partials + shared down bias, row max-abs normalize.
"""
import os
import sys
sys.path.insert(0, '/opt/trn_rl_repo')
import numpy as np
from contextlib import ExitStack

# debug knob: KMOE_SEQ=1 runs each PSUM accumulation group to completion
# before starting its sibling block (no A/B interleave)
SEQ_GROUPS = os.environ.get("KMOE_SEQ", "0") == "1"

H = 1024
I = 4096
E = 8
TOPK = 2
B, S = 2, 1024
T = B * S            # 2048 tokens
CAP = 544            # routed-token capacity per expert core (max count is 542)
BLK = CAP // 2       # 272-column blocks (<=512 fp32 PSUM bank limit)
HC = H // 128        # 8 h-chunks
IC = I // 128        # 32 i-chunks (routed experts)
ISL = I // E         # 512: shared-expert intermediate slice per core
ICS = ISL // 128     # 4 i-chunks (shared slice)
TB = 512             # shared-expert token block
NTP = T // (2 * TB)  # 2 token-block pairs

_COMPILED = {}


def _build_nc():
    from concourse import bacc, tile, mybir

    F32 = mybir.dt.float32
    CDT = mybir.dt.bfloat16
    GELU = mybir.ActivationFunctionType.Gelu
    IDENT = mybir.ActivationFunctionType.Identity

    nc = bacc.Bacc("TRN2", target_bir_lowering=False, debug=False, num_devices=E)

    xT_d = nc.dram_tensor("xT", [128, HC * CAP], CDT, kind="ExternalInput")
    xa_d = nc.dram_tensor("xa", [128, HC * T], CDT, kind="ExternalInput")
    upw_d = nc.dram_tensor("upw", [128, IC * HC * 128], CDT, kind="ExternalInput")
    dnw_d = nc.dram_tensor("dnw", [128, HC * IC * 128], CDT, kind="ExternalInput")
    suw_d = nc.dram_tensor("suw", [128, ICS * HC * 128], CDT, kind="ExternalInput")
    sdw_d = nc.dram_tensor("sdw", [128, HC * ICS * 128], CDT, kind="ExternalInput")
    upb_d = nc.dram_tensor("upb", [128, IC], F32, kind="ExternalInput")
    supb_d = nc.dram_tensor("supb", [128, ICS], F32, kind="ExternalInput")
    dnb_d = nc.dram_tensor("dnb", [128, HC], F32, kind="ExternalInput")
    eoT_d = nc.dram_tensor("eoT", [HC, 128, CAP], F32, kind="ExternalOutput")
    spT_d = nc.dram_tensor("spT", [HC, 128, T], CDT, kind="ExternalOutput")

    with tile.TileContext(nc) as tc, ExitStack() as ctx:
        pool = ctx.enter_context(tc.tile_pool(name="persist", bufs=1))
        wup = ctx.enter_context(tc.tile_pool(name="wup", bufs=3))
        wdn = ctx.enter_context(tc.tile_pool(name="wdn", bufs=3))
        hr = ctx.enter_context(tc.tile_pool(name="hr", bufs=IC))
        hsp = ctx.enter_context(tc.tile_pool(name="hsp", bufs=ICS))
        orp = ctx.enter_context(tc.tile_pool(name="orp", bufs=3))
        osp = ctx.enter_context(tc.tile_pool(name="osp", bufs=3))
        pup = ctx.enter_context(tc.tile_pool(name="pup", bufs=2, space="PSUM"))
        pdn = ctx.enter_context(tc.tile_pool(name="pdn", bufs=2, space="PSUM"))

        # ---- input DMAs, ordered for the startup ramp: the first up-matmul
        # group needs xT + the first up-weight i-chunks only; everything the
        # shared phase needs is deferred past the routed-up weight stream.
        xT_t = pool.tile([128, HC * CAP], CDT, tag="xT")
        xhalf = HC * CAP // 2
        nc.sync.dma_start(xT_t[:, :xhalf], xT_d.ap()[:, :xhalf])
        nc.sync.dma_start(xT_t[:, xhalf:], xT_d.ap()[:, xhalf:])

        uw_tiles = []
        uw0 = wup.tile([128, 4 * HC * 128], CDT, tag="uw")
        nc.sync.dma_start(uw0[:, :2048], upw_d.ap()[:, 0:2048])
        nc.sync.dma_start(uw0[:, 2048:], upw_d.ap()[:, 2048:4096])
        uw_tiles.append(uw0)
        upb_t = pool.tile([128, IC], F32, tag="upb")
        nc.sync.dma_start(upb_t[:], upb_d.ap()[:])
        for g in (1, 2):
            uwg = wup.tile([128, 4 * HC * 128], CDT, tag="uw", name=f"uwp{g}")
            nc.sync.dma_start(uwg[:], upw_d.ap()[:, g * 4096:(g + 1) * 4096])
            uw_tiles.append(uwg)

        # PE warm-up: dummy matmuls on a zeroed tile keep the PE busy through
        # the HAM activity window while the first weights stream in, so the
        # real matmul stream starts at the full clock.  Paced to end just
        # before the first real matmul (~13us): 26 x N=256 cold ~ 5.5us.
        wz_t = pool.tile([128, 256], CDT, tag="wz")
        nc.gpsimd.memset(wz_t[:], 0.0)
        psW = pup.tile([128, 512], F32, tag="uA")
        for _ in range(26):
            nc.tensor.matmul(psW[:, :256], wz_t[:, :128], wz_t[:], start=True, stop=True)

        def mm_pair(psA, psB, wfn, rA, rB, n):
            """Two accumulation groups over n contraction chunks."""
            if SEQ_GROUPS:
                for k in range(n):
                    nc.tensor.matmul(psA, wfn(k), rA(k), start=(k == 0), stop=(k == n - 1))
                for k in range(n):
                    nc.tensor.matmul(psB, wfn(k), rB(k), start=(k == 0), stop=(k == n - 1))
            else:
                for k in range(n):
                    w = wfn(k)
                    nc.tensor.matmul(psA, w, rA(k), start=(k == 0), stop=(k == n - 1))
                    nc.tensor.matmul(psB, w, rB(k), start=(k == 0), stop=(k == n - 1))

        # ---- routed up: hact[ic] = gelu(up_w.T @ x + b), two 272-col blocks
        hacts = []
        for icx in range(IC):
            g, off = divmod(icx, 4)
            if off == 0 and g >= 3:
                uwg = wup.tile([128, 4 * HC * 128], CDT, tag="uw", name=f"uwg{g}")
                nc.sync.dma_start(uwg[:], upw_d.ap()[:, g * 4096:(g + 1) * 4096])
                uw_tiles.append(uwg)
            uwt = uw_tiles[g]
            psA = pup.tile([128, 512], F32, tag="uA")
            psB = pup.tile([128, 512], F32, tag="uB")
            ht = hr.tile([128, CAP], CDT, tag="hact")
            mm_pair(psA[:, :BLK], psB[:, :BLK],
                    lambda hc, uwt=uwt, off=off: uwt[:, off * 1024 + hc * 128: off * 1024 + (hc + 1) * 128],
                    lambda hc: xT_t[:, hc * CAP: hc * CAP + BLK],
                    lambda hc: xT_t[:, hc * CAP + BLK: (hc + 1) * CAP], HC)
            nc.scalar.activation(ht[:, :BLK], psA[:, :BLK], GELU, bias=upb_t[:, icx:icx + 1])
            nc.scalar.activation(ht[:, BLK:], psB[:, :BLK], GELU, bias=upb_t[:, icx:icx + 1])
            hacts.append(ht)

        # shared-phase inputs (needed only ~100us later).  The scheduler
        # hoists dependency-free DMAs into the startup ramp where they
        # starve the critical xT/up-weight transfers, so gate each on an
        # early hact tile via a 1-column dummy copy (WAW on the DMA dest):
        # the transfers then start a few ic-groups into the routed-up phase.
        xa_t = pool.tile([128, HC * T], CDT, tag="xa")
        half = HC * T // 2
        nc.vector.tensor_copy(xa_t[:, :1], hacts[4][:, :1])
        nc.vector.tensor_copy(xa_t[:, half:half + 1], hacts[4][:, 1:2])
        nc.sync.dma_start(xa_t[:, :half], xa_d.ap()[:, :half])
        nc.sync.dma_start(xa_t[:, half:], xa_d.ap()[:, half:])
        suw_t = pool.tile([128, ICS * HC * 128], CDT, tag="suw")
        nc.vector.tensor_copy(suw_t[:, :1], hacts[6][:, :1])
        nc.sync.dma_start(suw_t[:], suw_d.ap()[:])
        sdw_t = pool.tile([128, HC * ICS * 128], CDT, tag="sdw")
        nc.vector.tensor_copy(sdw_t[:, :1], hacts[8][:, :1])
        nc.sync.dma_start(sdw_t[:], sdw_d.ap()[:])
        supb_t = pool.tile([128, ICS], F32, tag="supb")
        nc.sync.dma_start(supb_t[:], supb_d.ap()[:])
        dnb_t = pool.tile([128, HC], F32, tag="dnb")
        nc.sync.dma_start(dnb_t[:], dnb_d.ap()[:])

        # ---- routed down: eoT[hb] = 0.1 * (dn_w.T @ hact + b)
        for hb in range(HC):
            dwt = wdn.tile([128, IC * 128], CDT, tag="dw")
            nc.sync.dma_start(dwt[:], dnw_d.ap()[:, hb * IC * 128:(hb + 1) * IC * 128])
            psA = pdn.tile([128, 512], F32, tag="dA")
            psB = pdn.tile([128, 512], F32, tag="dB")
            mm_pair(psA[:, :BLK], psB[:, :BLK],
                    lambda icx, dwt=dwt: dwt[:, icx * 128:(icx + 1) * 128],
                    lambda icx: hacts[icx][:, :BLK],
                    lambda icx: hacts[icx][:, BLK:], IC)
            ot = orp.tile([128, CAP], F32, tag="or")
            nc.scalar.activation(ot[:, :BLK], psA[:, :BLK], IDENT,
                                 bias=dnb_t[:, hb:hb + 1], scale=0.1)
            nc.scalar.activation(ot[:, BLK:], psB[:, :BLK], IDENT,
                                 bias=dnb_t[:, hb:hb + 1], scale=0.1)
            nc.sync.dma_start(eoT_d.ap()[hb, :, :], ot[:])

        # ---- shared up: hs[ic] = gelu(suw_slice.T @ x_all + b), N=512 blocks
        hs_tiles = []
        for icx in range(ICS):
            ht = hsp.tile([128, T], CDT, tag="hs")
            for tp in range(NTP):
                psA = pup.tile([128, 512], F32, tag="uA")
                psB = pup.tile([128, 512], F32, tag="uB")
                mm_pair(psA[:], psB[:],
                        lambda hc, icx=icx: suw_t[:, icx * 1024 + hc * 128: icx * 1024 + (hc + 1) * 128],
                        lambda hc, tp=tp: xa_t[:, hc * T + tp * 2 * TB: hc * T + tp * 2 * TB + TB],
                        lambda hc, tp=tp: xa_t[:, hc * T + tp * 2 * TB + TB: hc * T + (tp + 1) * 2 * TB], HC)
                o0 = tp * 2 * TB
                nc.scalar.activation(ht[:, o0: o0 + TB], psA[:], GELU,
                                     bias=supb_t[:, icx:icx + 1])
                nc.scalar.activation(ht[:, o0 + TB: o0 + 2 * TB], psB[:], GELU,
                                     bias=supb_t[:, icx:icx + 1])
            hs_tiles.append(ht)

        # ---- shared down: spT[hb] = 0.1 * (sdw_slice.T @ hs)  (bias on host)
        for hb in range(HC):
            for tp in range(NTP):
                psA = pdn.tile([128, 512], F32, tag="dA")
                psB = pdn.tile([128, 512], F32, tag="dB")
                mm_pair(psA[:], psB[:],
                        lambda icx, hb=hb: sdw_t[:, hb * ICS * 128 + icx * 128: hb * ICS * 128 + (icx + 1) * 128],
                        lambda icx, tp=tp: hs_tiles[icx][:, tp * 2 * TB: tp * 2 * TB + TB],
                        lambda icx, tp=tp: hs_tiles[icx][:, tp * 2 * TB + TB: (tp + 1) * 2 * TB], ICS)
                ot = osp.tile([128, 2 * TB], CDT, tag="os")
                nc.scalar.activation(ot[:, :TB], psA[:], IDENT, scale=0.1)
                nc.sync.dma_start(spT_d.ap()[hb, :, tp * 2 * TB: tp * 2 * TB + TB],
                                  ot[:, :TB])
                nc.scalar.activation(ot[:, TB:], psB[:], IDENT, scale=0.1)
                nc.sync.dma_start(spT_d.ap()[hb, :, tp * 2 * TB + TB:(tp + 1) * 2 * TB],
                                  ot[:, TB:])

    nc.compile()
    return nc


def _get_compiled():
    if "nc" not in _COMPILED:
        _COMPILED["nc"] = _build_nc()
    return _COMPILED["nc"]


def _np_cdt():
    import ml_dtypes
    return np.dtype(ml_dtypes.bfloat16)


def _pack_weight(w):
    """[K, N] -> [128, (N/128 chunks) x (K/128 subtiles) x 128] stream layout."""
    kdim, ndim = w.shape
    kc, nchunk = kdim // 128, ndim // 128
    return np.ascontiguousarray(
        w.reshape(kc, 128, nchunk, 128).transpose(1, 2, 0, 3)
    ).reshape(128, nchunk * kc * 128).astype(_np_cdt())


def _pack_tokens(xsel, cap):
    """[n, H] tokens -> [128, HC*cap] transposed h-chunked layout, zero pad."""
    n = xsel.shape[0]
    arr = np.zeros((128, HC, cap), np.float32)
    if n:
        arr[:, :, :n] = xsel.T.reshape(HC, 128, n).transpose(1, 0, 2)
    return np.ascontiguousarray(arr).reshape(128, HC * cap).astype(_np_cdt())


def _pack_bias(b, scale=1.0):
    """[N] -> [128, N/128] per-partition layout."""
    return np.ascontiguousarray(
        (np.asarray(b, np.float32) * scale).reshape(-1, 128).T.astype(np.float32))


def kernel(x, gate_w, bias, up_w, up_b, down_w, down_b,
           sw_up, sb_up, sw_down, sb_down):
    from concourse.bass_utils import run_bass_kernel_spmd

    x = np.asarray(x, np.float32)
    xf = x.reshape(T, H)

    # ---- host gating (fp64 scores for a stable top-k, fp32 combine weights)
    z64 = xf.astype(np.float64) @ np.asarray(gate_w, np.float64) \
        + np.asarray(bias, np.float64)
    scores64 = 1.0 / (1.0 + np.exp(-z64))
    top_idx = np.argsort(-scores64, axis=-1, kind="stable")[:, :TOPK]
    tsc = scores64[np.arange(T)[:, None], top_idx].astype(np.float32)
    wts = tsc / (tsc.sum(-1, keepdims=True) + np.float32(1e-6))   # [T, 2]

    # ---- token dispatch
    tok_lists = [np.where((top_idx == e).any(-1))[0] for e in range(E)]
    for e, tl in enumerate(tok_lists):
        if len(tl) > CAP:
            raise RuntimeError(f"expert {e} overflow: {len(tl)} > CAP={CAP}")

    xa = _pack_tokens(xf, T)
    in_maps = []
    for e in range(E):
        in_maps.append({
            "xT": _pack_tokens(xf[tok_lists[e]], CAP),
            "xa": xa,
            "upw": _pack_weight(np.asarray(up_w[e], np.float32)),
            "dnw": _pack_weight(np.asarray(down_w[e], np.float32)),
            "suw": _pack_weight(np.asarray(sw_up[:, e * ISL:(e + 1) * ISL], np.float32)),
            "sdw": _pack_weight(np.asarray(sw_down[e * ISL:(e + 1) * ISL, :], np.float32)),
            "upb": _pack_bias(up_b[e]),
            "supb": _pack_bias(np.asarray(sb_up, np.float32)[e * ISL:(e + 1) * ISL]),
            "dnb": _pack_bias(down_b[e], scale=0.1),
        })

    nc = _get_compiled()
    res = run_bass_kernel_spmd(nc, in_maps, list(range(E)))

    # ---- host combine: shared partial sum + bias, expert scatter-add, normalize
    sp = np.zeros((H, T), np.float32)
    for e in range(E):
        sp += np.asarray(res.results[e]["spT"], np.float32).reshape(H, T)
    out = sp.T + np.float32(0.1) * np.asarray(sb_down, np.float32)[None, :]

    for e in range(E):
        tl = tok_lists[e]
        if len(tl) == 0:
            continue
        eoT = np.asarray(res.results[e]["eoT"], np.float32)   # [HC, 128, CAP]
        eo = eoT.reshape(H, CAP)[:, :len(tl)].T               # [n, H]
        we = np.where(top_idx[tl, 0] == e, wts[tl, 0], wts[tl, 1]).astype(np.float32)
        out[tl] += we[:, None] * eo

    out /= (np.abs(out).max(-1, keepdims=True) + np.float32(1e-6))
    return out.reshape(B, S, H)
